# revision 36
# baseline (speedup 1.0000x reference)
"""Trainium2 Bass kernel for nn_Attention_Param_sharing_Kv_sharing.

Reference computation (per batch b, with x_b = x[b] viewed as [C=256, N=4096]):
    K   = w_qk' @ x_b + t_qk                  [16, N]    (BN folded into w', t)
    S   = K^T K                               [N, N]     (q == k shared -> symmetric)
    P   = exp(S)        (no max-subtraction; |S| < ~40 so fp32 exp is safe)
    r   = row sums of P = column sums of P    (symmetry)
    XXu^T[c,n] = sum_m V[c,m] P[m,n]          (= (attn @ V) * r, pre-normalized)
    out = (w_p' @ relu(XXu^T) + t_p (x) r) * (1/r)       [256, N]

Sharding: 8 cores = 4 batches x 2 column-halves of N.  The host permutes the
spatial axis per core so each core's own 2048 columns come first (attention
is permutation-equivariant over m when K and V are permuted together, and r
is permutation-invariant), which keeps the device program SPMD-uniform.

The device returns the unnormalized projection O_u = w_p' @ relu(XXu^T)
plus per-ROW sums R[m] = sum_n P[m, n-half].  R comes for free from the
Scalar engine's accumulate port (`accum_out`) on the exp instruction itself:
the n-loop runs in 1024-wide superblocks so each exp call covers exactly one
m-tile, making the per-instruction accumulator exactly R[m-tile] for that
superblock.  Because P is symmetric, the column sums r[n] are recovered on
the host as R_half0[n] + R_half1[n] from a batch's two half-slabs, and the
final (O_u + t_p (x) r) / r = O_u/r + t_p is a trivial elementwise host
epilogue (same class of host work as the BN folding / permutation already
done in make_in_maps).  This removes the 128 PE row-sum matmuls, the rank-1
t_p (x) r PSUM updates and the whole reciprocal/broadcast chain of the
previous version.

Symmetry of P means the P tiles computed in [m-partition, n-free] layout are
directly the P^T operand needed by the attn@V matmul -- no transposes.  P
tiles are consumed by attn@V immediately after exp, so only a small
round-robin window of them lives in SBUF.
"""

import numpy as np
import ml_dtypes

import concourse.bass as bass
import concourse.mybir as mybir
import concourse.tile as tile
from concourse import bacc
from concourse.bass import ts

F32 = mybir.dt.float32
F32R = mybir.dt.float32r
BF16 = mybir.dt.bfloat16

N_CORES = 8
B, C, H, W = 4, 256, 64, 64
N = H * W            # 4096
KD = 16              # qk dim
DH = 128             # value channels
EPS = 1e-5

NSH = N // 2         # 2048 n-columns per core
NSB = 1024           # n-superblock width (one exp call = one m-tile x NSB)
NSBLOCKS = NSH // NSB  # 2
NBLK = 512           # psum-bank chunk
MT = N // 128        # 32 m-tiles

_CACHE = {}


def _emit(nc, pools, dram, pack_s=True, probe=None, rep=0, deferred=None):
    const, work, outp, pgrp, ps_s, ps_xx, ps_pj = pools
    (xf_d, xb_d, wqkT_d, wvT_d, wpT_d, tqk_d, tv_d, out_d, r_d) = dram

    # ---- constants / weights ----
    # Input DMAs are split into N-quarter chunks and issued in consumption
    # order, so the K projection (and the first exp) starts after ~1MB of
    # traffic instead of the full 6MB input load.
    xf = const.tile([128, 2, N], F32R, tag="xf")
    xb = const.tile([128, 2, N], BF16, tag="xb")
    wqkT = const.tile([128, 2, 128], F32R, tag="wqkT")
    wvT = const.tile([128, 2, DH], BF16, tag="wvT")
    wpT = const.tile([128, 2, 128], F32R, tag="wpT")
    tqk = const.tile([128, 1], F32, tag="tqk")
    tvb = const.tile([128, DH], F32, tag="tvb")

    def chunk_dma(dst, src_d, i, w):
        nc.sync.dma_start(
            out=dst[:, :, ts(i, w)],
            in_=src_d.ap()[:, :, ts(i, w)],
        )

    nc.sync.dma_start(out=wqkT, in_=wqkT_d.ap())
    nc.sync.dma_start(out=tqk, in_=tqk_d.ap())
    chunk_dma(xf, xf_d, 0, 512)   # first eighth: unblocks K-proj chunk 0
    chunk_dma(xf, xf_d, 1, 512)
    nc.sync.dma_start(out=wvT, in_=wvT_d.ap())
    nc.sync.dma_start(
        out=tvb, in_=bass.AP(tensor=tv_d, offset=0, ap=[[0, 128], [1, DH]])
    )
    chunk_dma(xb, xb_d, 0, 1024)
    chunk_dma(xf, xf_d, 1, 1024)
    chunk_dma(xf, xf_d, 2, 1024)
    chunk_dma(xf, xf_d, 3, 1024)
    chunk_dma(xb, xb_d, 1, 1024)
    chunk_dma(xb, xb_d, 2, 1024)
    chunk_dma(xb, xb_d, 3, 1024)
    nc.sync.dma_start(out=wpT, in_=wpT_d.ap())
    # R[m] accumulators: one [128,1] column per (m-tile, superblock)
    r_sb = const.tile([128, MT, NSBLOCKS], F32, tag="r_sb")
    if rep == 0:
        # dummy exp: loads the ACT exp table set during the prologue instead
        # of stalling the first real exp call ~2.7us
        warm_sb = work.tile([1, 1], F32, tag="warm")
        nc.scalar.activation(
            out=warm_sb, in_=tqk[0:1, 0:1],
            func=mybir.ActivationFunctionType.Exp,
        )
        # warm-up matmuls: start the PE p-state ramp during the input DMA
        # so the first K-projection doesn't run at the cold clock
        for i in range(2):
            wps_full = ps_pj.tile([128, NBLK], F32, tag="pj", name="wps")
            nc.tensor.matmul(
                wps_full[:, 0:64], wqkT[:, 0, :], wqkT[:, 1, 0:64],
                start=True, stop=True,
            )

    # ---- K projection (replicated 4x across 32-row groups for S packing):
    # k_sb rows 32g+d (d<16) hold K[d, :]; rows 32g+16.. are zero.
    # 512-column chunks; chunks 0-1 are emitted up front (enough for the
    # first rounds), the rest are interleaved into superblock 0's round
    # loop so the in-order PE stream never parks behind a late xf DMA.  ----
    k_sb = const.tile([128, N], F32R, tag="k_sb")

    def emit_kproj(i):
        kps_full = ps_s.tile([128, NSB], F32, tag="s", name="kps")
        kps = kps_full[:, 0:NBLK]
        for cb in range(2):
            nc.tensor.matmul(
                kps,
                wqkT[:, cb, :],
                xf[:, cb, ts(i, NBLK)],
                start=(cb == 0),
                stop=(cb == 1),
            )
        nc.vector.tensor_scalar(
            out=k_sb[:, ts(i, NBLK)],
            in0=kps,
            scalar1=tqk,
            scalar2=None,
            op0=mybir.AluOpType.add,
        )

    for i in range(2):
        emit_kproj(i)

    # ---- V^T: VT[m, c] = sum_C x[C, m] wv'[c, C] + tv  -> bf16 ----
    # Only the first pair of m-tiles is computed up front; the rest are
    # emitted inside superblock 0's round loop (one round of lookahead) so
    # the scalar engine is already busy with exp while they run.
    vt_sb = const.tile([128, MT, DH], BF16, tag="vt_sb")

    def emit_vt(mi):
        vps_full = ps_pj.tile([128, NBLK], F32, tag="pj", name="vps")
        vps = vps_full[:, 0:DH]
        for cb in range(2):
            nc.tensor.matmul(
                vps,
                xb[:, cb, ts(mi, 128)],
                wvT[:, cb, :],
                start=(cb == 0),
                stop=(cb == 1),
            )
        nc.vector.tensor_add(vt_sb[:, mi, :], vps, tvb)

    for mi in range(2):
        emit_vt(mi)

    # ---- main loop over this core's n-superblocks ----
    # The epilogue for superblock J-1 is software-pipelined into J's round
    # loop so its DVE/PE work overlaps ACT's exp stream.
    def epilogue_pieces(st, tail=False):
        """Four pieces, one per (chunk, out-half), to be spread across
        successive exp shadows so no single shadow overflows."""
        J, xxp = st["J"], st["xx"]
        relus = {}

        def make(c, h2):
            def piece():
                if h2 == 0:
                    relu_sb = work.tile([128, NBLK], F32R, tag="relu")
                    if tail:
                        # ACT is idle once the exp stream ends
                        nc.scalar.activation(
                            out=relu_sb,
                            in_=xxp[c],
                            func=mybir.ActivationFunctionType.Relu,
                        )
                    else:
                        nc.vector.tensor_scalar(
                            out=relu_sb,
                            in0=xxp[c],
                            scalar1=0.0,
                            scalar2=None,
                            op0=mybir.AluOpType.max,
                        )
                    relus[c] = relu_sb
                pjps = ps_pj.tile([128, NBLK], F32, tag="pj")
                nc.tensor.matmul(
                    pjps, wpT[:, h2, :], relus[c], start=True, stop=True
                )
                o_sb = outp.tile([128, NBLK], F32, tag="o")
                nc.vector.tensor_copy(o_sb, pjps)
                nc.sync.dma_start(
                    out=out_d[h2, :, ts(J * 2 + c, NBLK)], in_=o_sb
                )
            return piece

        # relus (h2==0 pieces) first: they free the xx psum slots that the
        # next superblock's attn@V accumulation is waiting to reuse
        return [make(0, 0), make(1, 0), make(0, 1), make(1, 1)]

    # queue of deferred epilogue pieces to drip into upcoming exp shadows
    pieces = list(deferred) if deferred else []

    prev = None
    for J in range(NSBLOCKS):
        xxA = ps_xx.tile([128, NBLK], F32, tag="xx", name="xxA")
        xxB = ps_xx.tile([128, NBLK], F32, tag="xx", name="xxB")
        xxp = (xxA, xxB)

        # attn@V runs one m-tile behind exp, so the PE work gating the next
        # exp (its 2 S matmuls) plus the deferred attn@V of the previous
        # m-tile both fit inside the ACT shadow of the current exp.
        pend = None  # (p_sb, q, mi) owing its attn@V

        def emit_attnv(ent):
            pp, q, mi = ent
            for c in range(2):
                nc.tensor.matmul(
                    xxp[c],
                    vt_sb[:, mi, :],
                    pp[:, q, ts(c, NBLK)],
                    start=(mi == 0),
                    stop=(mi == MT - 1),
                )

        for t in range(MT // 2):  # rounds of 2 m-tiles
            p_sb = pgrp.tile([128, 2, NSB], BF16, tag="p")
            # Each exp is gated by only its own m-tile's 2 S matmuls; the
            # 4 S matmuls of a round are packed into distinct 32-row PE
            # groups.  exp's ACT accumulator gives R[m-tile] for free.
            for q in range(2):
                mi = 2 * t + q
                s_ps = ps_s.tile([128, NSB], F32, tag="s")
                for c in range(2):
                    g = 32 * (2 * q + c) if pack_s else 0
                    nc.tensor.matmul(
                        s_ps[:, ts(c, NBLK)],
                        k_sb[g:g + KD, ts(mi, 128)],
                        k_sb[g:g + KD, ts(J * 2 + c, NBLK)],
                        start=True,
                        stop=True,
                        tile_position=(g, 0),
                    )
                # drip one queued epilogue piece (previous superblock or
                # previous rep's tail) into this exp's shadow (q==1: the
                # V^T lookahead occupies the q==0 shadow)
                if pieces and q == 1:
                    pieces.pop(0)()
                nc.scalar.activation(
                    out=p_sb[:, q, :],
                    in_=s_ps,
                    func=mybir.ActivationFunctionType.Exp,
                    accum_out=r_sb[:, mi, J:J + 1],
                )
                if pend is not None:
                    emit_attnv(pend)
                pend = (p_sb, q, mi)
                # lookahead V^T / K-proj chunks (superblock 0 only), split
                # across the two exp shadows of the round
                if J == 0 and q == 0 and t < MT // 2 - 1:
                    emit_vt(2 * t + 2)
                    emit_vt(2 * t + 3)
                if J == 0 and q == 1 and t < 6:
                    emit_kproj(2 + t)
            # queue the previous superblock's epilogue pieces
            if prev is not None and t == 1:
                pieces.extend(epilogue_pieces(prev))
                prev = None
        emit_attnv(pend)  # flush: xx must close before this J's epilogue
        while pieces:  # anything not yet dripped (shouldn't happen)
            pieces.pop(0)()

        prev = {"J": J, "xx": xxp}

    # tail: epilogue for the last superblock + R write-out.  Returned as a
    # piece list so build_nc can defer it into the NEXT rep's emission
    # (cross-rep software pipelining); the final rep's pieces are emitted
    # at the program end with ACT relus (ACT idle there).
    last = prev

    def make_tail(tail):
        def r_piece():
            nc.sync.dma_start(out=r_d.ap(), in_=r_sb)
        return [r_piece] + epilogue_pieces(last, tail=tail)

    return make_tail


def build_nc(reps=1, pack_s=True, probe=None):
    key = ("nc", reps, pack_s, probe)
    if key in _CACHE:
        return _CACHE[key]

    nc = bacc.Bacc("TRN2", target_bir_lowering=False, debug=False)

    xf_d = nc.dram_tensor("xf", [128, 2, N], F32R, kind="ExternalInput")
    xb_d = nc.dram_tensor("xb", [128, 2, N], BF16, kind="ExternalInput")
    wqkT_d = nc.dram_tensor("wqkT", [128, 2, 128], F32R, kind="ExternalInput")
    wvT_d = nc.dram_tensor("wvT", [128, 2, DH], BF16, kind="ExternalInput")
    wpT_d = nc.dram_tensor("wpT", [128, 2, 128], F32R, kind="ExternalInput")
    tqk_d = nc.dram_tensor("tqk", [128, 1], F32, kind="ExternalInput")
    tv_d = nc.dram_tensor("tv", [1, DH], F32, kind="ExternalInput")
    out_d = nc.dram_tensor("out", [2, 128, NSH], F32, kind="ExternalOutput")
    r_d = nc.dram_tensor("r", [128, MT, NSBLOCKS], F32, kind="ExternalOutput")
    dram = (xf_d, xb_d, wqkT_d, wvT_d, wpT_d, tqk_d, tv_d, out_d, r_d)

    with tile.TileContext(nc) as tc:
        with (
            tc.tile_pool(name="const", bufs=1) as const,
            tc.tile_pool(name="work", bufs=3) as work,
            tc.tile_pool(name="outp", bufs=6) as outp,
            tc.tile_pool(name="pgrp", bufs=6) as pgrp,
            tc.tile_pool(name="ps_s", bufs=2, space="PSUM") as ps_s,
            tc.tile_pool(name="ps_xx", bufs=2, space="PSUM") as ps_xx,
            tc.tile_pool(name="ps_pj", bufs=2, space="PSUM") as ps_pj,
        ):
            pools = (const, work, outp, pgrp, ps_s, ps_xx, ps_pj)
            make_tail = None
            for rep in range(reps):
                deferred = make_tail(False) if make_tail is not None else None
                make_tail = _emit(nc, pools, dram, pack_s=pack_s, probe=probe,
                                  rep=rep, deferred=deferred)
            for piece in make_tail(True):
                piece()

    nc.compile()
    _CACHE[key] = nc
    return nc


def fold_bn(w, g, b, m, v):
    s = (g / np.sqrt(v + EPS)).astype(np.float32)
    return (w * s[:, None]).astype(np.float32), (b - m * s).astype(np.float32)


def make_in_maps(x, w_qk, g_qk, b_qk, m_qk, v_qk,
                 w_v, g_v, b_v, m_v, v_v, w_p, g_p, b_p, m_p, v_p):
    wqk_f, tqk_f = fold_bn(w_qk, g_qk, b_qk, m_qk, v_qk)   # [16,256], [16]
    wv_f, tv_f = fold_bn(w_v, g_v, b_v, m_v, v_v)          # [128,256], [128]
    wp_f, tp_f = fold_bn(w_p, g_p, b_p, m_p, v_p)          # [256,128], [256]

    # [128, 2, *]: partition dim first, C-half (or out-half) second.
    # wqkT replicated into 4 column groups of 32 (16 used + 16 zero) so the
    # S stage can row-pack 4 concurrent matmuls.
    wqkT_h = wqk_f.T.reshape(2, 128, KD).transpose(1, 0, 2)  # [128, 2, 16]
    wqkT = np.zeros((128, 2, 128), np.float32)
    for g in range(4):
        wqkT[:, :, 32 * g:32 * g + KD] = wqkT_h
    wqkT = np.ascontiguousarray(wqkT)
    wvT = np.ascontiguousarray(
        wv_f.T.reshape(2, 128, DH).transpose(1, 0, 2)).astype(ml_dtypes.bfloat16)
    wpT = np.ascontiguousarray(
        wp_f.T.reshape(128, 2, 128)).astype(np.float32)
    tqk = np.zeros((128, 1), np.float32)
    for g in range(4):
        tqk[32 * g:32 * g + KD, 0] = tqk_f
    tqk = np.ascontiguousarray(tqk)
    tv = tv_f.reshape(1, DH).astype(np.float32)

    xr = x.reshape(B, C, N).astype(np.float32)
    in_maps = []
    for c in range(N_CORES):
        b_, h_ = c // 2, c % 2
        # permute n so this core's half comes first
        if h_ == 0:
            xp = xr[b_]
        else:
            xp = np.concatenate([xr[b_][:, NSH:], xr[b_][:, :NSH]], axis=1)
        xp = np.ascontiguousarray(xp.reshape(2, 128, N).transpose(1, 0, 2))
        in_maps.append({
            "xf": xp.astype(np.float32),
            "xb": xp.astype(ml_dtypes.bfloat16),
            "wqkT": wqkT, "wvT": wvT, "wpT": wpT,
            "tqk": tqk, "tv": tv,
        })
    return in_maps, tp_f


def assemble(results, tp_f):
    """Per-core 'out' [2,128,NSH] + 'r' [128,MT,NSBLOCKS] -> full [B,C,H,W].

    r[n] = R_half0[n] + R_half1[n] by symmetry of P; final epilogue
    out = O_u / r + t_p (relu commutes with the positive 1/r scale, so the
    device applied w_p' @ relu() unnormalized).
    """
    out = np.empty((B, C, N), np.float32)
    for b_ in range(B):
        rv = []
        for h_ in range(2):
            rr = results[2 * b_ + h_]["r"].astype(np.float32)  # [128,MT,2]
            rl = rr.sum(axis=2).T.reshape(-1)  # R[m], local (permuted) order
            if h_ == 1:
                rl = np.concatenate([rl[NSH:], rl[:NSH]])      # unpermute
            rv.append(rl)
        r = rv[0] + rv[1]                                      # [N]
        for h_ in range(2):
            o = results[2 * b_ + h_]["out"].reshape(C, NSH)
            sl = slice(h_ * NSH, (h_ + 1) * NSH)
            out[b_][:, sl] = o / r[sl][None, :] + tp_f[:, None]
    return out.reshape(B, C, H, W)


def kernel(**inputs):
    from concourse.bass_utils import run_bass_kernel_spmd
    from concourse.bass_interp import get_hw_module

    inputs = {k: np.asarray(v) for k, v in inputs.items()}
    inputs.pop("key_v_input_reduction", None)  # unused by the reference
    nc = build_nc()
    in_maps, tp_f = make_in_maps(**inputs)
    old_m = nc.m
    nc.m = get_hw_module(nc.m)
    try:
        res = run_bass_kernel_spmd(nc, in_maps, core_ids=list(range(N_CORES)))
    finally:
        nc.m = old_m
    return assemble(res.results, tp_f)


# revision 38
# speedup vs baseline: 5.5780x; 5.5780x over previous
"""Trainium2 Bass kernel for nn_Attention_Param_sharing_Kv_sharing.

Reference computation (per batch b, with x_b = x[b] viewed as [C=256, N=4096]):
    K   = w_qk' @ x_b + t_qk                  [16, N]    (BN folded into w', t)
    S   = K^T K                               [N, N]     (q == k shared -> symmetric)
    P   = exp(S)        (no max-subtraction; |S| < ~40 so fp32 exp is safe)
    r   = row sums of P = column sums of P    (symmetry)
    XXu^T[c,n] = sum_m V[c,m] P[m,n]          (= (attn @ V) * r, pre-normalized)
    out = (w_p' @ relu(XXu^T) + t_p (x) r) * (1/r)       [256, N]

Sharding: 8 cores = 4 batches x 2 column-halves of N.  The host permutes the
spatial axis per core so each core's own 2048 columns come first (attention
is permutation-equivariant over m when K and V are permuted together, and r
is permutation-invariant), which keeps the device program SPMD-uniform.

The device returns the unnormalized projection O_u = w_p' @ relu(XXu^T)
plus per-ROW sums R[m] = sum_n P[m, n-half].  R comes for free from the
Scalar engine's accumulate port (`accum_out`) on the exp instruction itself:
the n-loop runs in 1024-wide superblocks so each exp call covers exactly one
m-tile, making the per-instruction accumulator exactly R[m-tile] for that
superblock.  Because P is symmetric, the column sums r[n] are recovered on
the host as R_half0[n] + R_half1[n] from a batch's two half-slabs, and the
final (O_u + t_p (x) r) / r = O_u/r + t_p is a trivial elementwise host
epilogue (same class of host work as the BN folding / permutation already
done in make_in_maps).  This removes the 128 PE row-sum matmuls, the rank-1
t_p (x) r PSUM updates and the whole reciprocal/broadcast chain of the
previous version.

Symmetry of P means the P tiles computed in [m-partition, n-free] layout are
directly the P^T operand needed by the attn@V matmul -- no transposes.  P
tiles are consumed by attn@V immediately after exp, so only a small
round-robin window of them lives in SBUF.
"""

import numpy as np
import ml_dtypes

import concourse.bass as bass
import concourse.mybir as mybir
import concourse.tile as tile
from concourse import bacc
from concourse.bass import ts

F32 = mybir.dt.float32
F32R = mybir.dt.float32r
BF16 = mybir.dt.bfloat16

N_CORES = 8
B, C, H, W = 4, 256, 64, 64
N = H * W            # 4096
KD = 16              # qk dim
DH = 128             # value channels
EPS = 1e-5

NSH = N // 2         # 2048 n-columns per core
NSB = 1024           # n-superblock width (one exp call = one m-tile x NSB)
NSBLOCKS = NSH // NSB  # 2
NBLK = 512           # psum-bank chunk
MT = N // 128        # 32 m-tiles

_CACHE = {}


def _emit(nc, pools, dram, pack_s=True, probe=None, rep=0, deferred=None):
    const, work, outp, pgrp, ps_s, ps_xx, ps_pj = pools
    (xf_d, xb_d, wqkT_d, wvT_d, wpT_d, tqk_d, tv_d, out_d, r_d) = dram

    # ---- constants / weights ----
    # Input DMAs are split into N-quarter chunks and issued in consumption
    # order, so the K projection (and the first exp) starts after ~1MB of
    # traffic instead of the full 6MB input load.
    xf = const.tile([128, 2, N], F32R, tag="xf")
    xb = const.tile([128, 2, N], BF16, tag="xb")
    wqkT = const.tile([128, 2, 128], F32R, tag="wqkT")
    wvT = const.tile([128, 2, DH], BF16, tag="wvT")
    wpT = const.tile([128, 2, 128], F32R, tag="wpT")
    tqk = const.tile([128, 1], F32, tag="tqk")
    tvb = const.tile([128, DH], F32, tag="tvb")

    def chunk_dma(dst, src_d, i, w):
        nc.sync.dma_start(
            out=dst[:, :, ts(i, w)],
            in_=src_d.ap()[:, :, ts(i, w)],
        )

    nc.sync.dma_start(out=wqkT, in_=wqkT_d.ap())
    nc.sync.dma_start(out=tqk, in_=tqk_d.ap())
    chunk_dma(xf, xf_d, 0, 512)   # first eighth: unblocks K-proj chunk 0
    chunk_dma(xf, xf_d, 1, 512)
    nc.sync.dma_start(out=wvT, in_=wvT_d.ap())
    nc.sync.dma_start(
        out=tvb, in_=bass.AP(tensor=tv_d, offset=0, ap=[[0, 128], [1, DH]])
    )
    chunk_dma(xb, xb_d, 0, 1024)
    chunk_dma(xf, xf_d, 1, 1024)
    chunk_dma(xf, xf_d, 2, 1024)
    chunk_dma(xf, xf_d, 3, 1024)
    chunk_dma(xb, xb_d, 1, 1024)
    chunk_dma(xb, xb_d, 2, 1024)
    chunk_dma(xb, xb_d, 3, 1024)
    nc.sync.dma_start(out=wpT, in_=wpT_d.ap())
    # R[m] accumulators: one [128,1] column per (m-tile, superblock)
    r_sb = const.tile([128, MT, NSBLOCKS], F32, tag="r_sb")
    if rep == 0:
        # dummy exp: loads the ACT exp table set during the prologue instead
        # of stalling the first real exp call ~2.7us
        warm_sb = work.tile([1, 1], F32, tag="warm")
        nc.scalar.activation(
            out=warm_sb, in_=tqk[0:1, 0:1],
            func=mybir.ActivationFunctionType.Exp,
        )
        # warm-up matmuls: start the PE p-state ramp during the input DMA
        # so the first K-projection doesn't run at the cold clock
        for i in range(2):
            wps_full = ps_pj.tile([128, NBLK], F32, tag="pj", name="wps")
            nc.tensor.matmul(
                wps_full[:, 0:64], wqkT[:, 0, :], wqkT[:, 1, 0:64],
                start=True, stop=True,
            )

    # ---- K projection (replicated 4x across 32-row groups for S packing):
    # k_sb rows 32g+d (d<16) hold K[d, :]; rows 32g+16.. are zero.
    # 512-column chunks; chunks 0-1 are emitted up front (enough for the
    # first rounds), the rest are interleaved into superblock 0's round
    # loop so the in-order PE stream never parks behind a late xf DMA.  ----
    k_sb = const.tile([128, N], F32R, tag="k_sb")

    def emit_kproj(i):
        kps_full = ps_s.tile([128, NSB], F32, tag="s", name="kps")
        kps = kps_full[:, 0:NBLK]
        for cb in range(2):
            nc.tensor.matmul(
                kps,
                wqkT[:, cb, :],
                xf[:, cb, ts(i, NBLK)],
                start=(cb == 0),
                stop=(cb == 1),
            )
        nc.vector.tensor_scalar(
            out=k_sb[:, ts(i, NBLK)],
            in0=kps,
            scalar1=tqk,
            scalar2=None,
            op0=mybir.AluOpType.add,
        )

    for i in range(2):
        emit_kproj(i)

    # ---- V^T: VT[m, c] = sum_C x[C, m] wv'[c, C] + tv  -> bf16 ----
    # Only the first pair of m-tiles is computed up front; the rest are
    # emitted inside superblock 0's round loop (one round of lookahead) so
    # the scalar engine is already busy with exp while they run.
    vt_sb = const.tile([128, MT, DH], BF16, tag="vt_sb")

    def emit_vt(mi):
        vps_full = ps_pj.tile([128, NBLK], F32, tag="pj", name="vps")
        vps = vps_full[:, 0:DH]
        for cb in range(2):
            nc.tensor.matmul(
                vps,
                xb[:, cb, ts(mi, 128)],
                wvT[:, cb, :],
                start=(cb == 0),
                stop=(cb == 1),
            )
        nc.vector.tensor_add(vt_sb[:, mi, :], vps, tvb)

    for mi in range(2):
        emit_vt(mi)

    # ---- main loop over this core's n-superblocks ----
    # The epilogue for superblock J-1 is software-pipelined into J's round
    # loop so its DVE/PE work overlaps ACT's exp stream.
    def epilogue_pieces(st, tail=False):
        """Four pieces, one per (chunk, out-half), to be spread across
        successive exp shadows so no single shadow overflows."""
        J, xxp = st["J"], st["xx"]
        relus = {}

        def make(c, h2):
            def piece():
                if h2 == 0:
                    relu_sb = work.tile([128, NBLK], F32R, tag="relu")
                    if tail:
                        # ACT is idle once the exp stream ends
                        nc.scalar.activation(
                            out=relu_sb,
                            in_=xxp[c],
                            func=mybir.ActivationFunctionType.Relu,
                        )
                    else:
                        nc.vector.tensor_scalar(
                            out=relu_sb,
                            in0=xxp[c],
                            scalar1=0.0,
                            scalar2=None,
                            op0=mybir.AluOpType.max,
                        )
                    relus[c] = relu_sb
                pjps = ps_pj.tile([128, NBLK], F32, tag="pj")
                nc.tensor.matmul(
                    pjps, wpT[:, h2, :], relus[c], start=True, stop=True
                )
                o_sb = outp.tile([128, NBLK], F32, tag="o")
                nc.vector.tensor_copy(o_sb, pjps)
                nc.sync.dma_start(
                    out=out_d[h2, :, ts(J * 2 + c, NBLK)], in_=o_sb
                )
            return piece

        # relus (h2==0 pieces) first: they free the xx psum slots that the
        # next superblock's attn@V accumulation is waiting to reuse
        return [make(0, 0), make(1, 0), make(0, 1), make(1, 1)]

    # queue of deferred epilogue pieces to drip into upcoming exp shadows
    pieces = list(deferred) if deferred else []

    prev = None
    for J in range(NSBLOCKS):
        xxA = ps_xx.tile([128, NBLK], F32, tag="xx", name="xxA")
        xxB = ps_xx.tile([128, NBLK], F32, tag="xx", name="xxB")
        xxp = (xxA, xxB)

        # attn@V runs one m-tile behind exp, so the PE work gating the next
        # exp (its 2 S matmuls) plus the deferred attn@V of the previous
        # m-tile both fit inside the ACT shadow of the current exp.
        pend = None  # (p_sb, q, mi) owing its attn@V

        def emit_attnv(ent):
            pp, q, mi = ent
            for c in range(2):
                nc.tensor.matmul(
                    xxp[c],
                    vt_sb[:, mi, :],
                    pp[:, q, ts(c, NBLK)],
                    start=(mi == 0),
                    stop=(mi == MT - 1),
                )

        for t in range(MT // 2):  # rounds of 2 m-tiles
            p_sb = pgrp.tile([128, 2, NSB], BF16, tag="p")
            # Each exp is gated by only its own m-tile's 2 S matmuls; the
            # 4 S matmuls of a round are packed into distinct 32-row PE
            # groups.  exp's ACT accumulator gives R[m-tile] for free.
            for q in range(2):
                mi = 2 * t + q
                s_ps = ps_s.tile([128, NSB], F32, tag="s")
                for c in range(2):
                    g = 32 * (2 * q + c) if pack_s else 0
                    nc.tensor.matmul(
                        s_ps[:, ts(c, NBLK)],
                        k_sb[g:g + KD, ts(mi, 128)],
                        k_sb[g:g + KD, ts(J * 2 + c, NBLK)],
                        start=True,
                        stop=True,
                        tile_position=(g, 0),
                    )
                # drip one queued epilogue piece (previous superblock or
                # previous rep's tail) into this exp's shadow (q==1: the
                # V^T lookahead occupies the q==0 shadow)
                if probe == "inline":
                    if pieces and t == 2 and q == 0:
                        while pieces:
                            pieces.pop(0)()
                elif pieces and q == 1:
                    pieces.pop(0)()
                nc.scalar.activation(
                    out=p_sb[:, q, :],
                    in_=s_ps,
                    func=mybir.ActivationFunctionType.Exp,
                    accum_out=r_sb[:, mi, J:J + 1],
                )
                if pend is not None:
                    emit_attnv(pend)
                pend = (p_sb, q, mi)
                # lookahead V^T / K-proj chunks (superblock 0 only), split
                # across the two exp shadows of the round
                if J == 0 and q == 0 and t < MT // 2 - 1:
                    emit_vt(2 * t + 2)
                    emit_vt(2 * t + 3)
                if J == 0 and q == 1 and t < 6:
                    emit_kproj(2 + t)
            # queue the previous superblock's epilogue pieces
            if prev is not None and t == 1:
                pieces.extend(epilogue_pieces(prev))
                prev = None
        emit_attnv(pend)  # flush: xx must close before this J's epilogue
        while pieces:  # anything not yet dripped (shouldn't happen)
            pieces.pop(0)()

        prev = {"J": J, "xx": xxp}

    # tail: epilogue for the last superblock + R write-out.  Returned as a
    # piece list so build_nc can defer it into the NEXT rep's emission
    # (cross-rep software pipelining); the final rep's pieces are emitted
    # at the program end with ACT relus (ACT idle there).
    last = prev

    def make_tail(tail):
        def r_piece():
            nc.sync.dma_start(out=r_d.ap(), in_=r_sb)
        return [r_piece] + epilogue_pieces(last, tail=tail)

    return make_tail


def build_nc(reps=1, pack_s=True, probe=None):
    key = ("nc", reps, pack_s, probe)
    if key in _CACHE:
        return _CACHE[key]

    nc = bacc.Bacc("TRN2", target_bir_lowering=False, debug=False)

    xf_d = nc.dram_tensor("xf", [128, 2, N], F32R, kind="ExternalInput")
    xb_d = nc.dram_tensor("xb", [128, 2, N], BF16, kind="ExternalInput")
    wqkT_d = nc.dram_tensor("wqkT", [128, 2, 128], F32R, kind="ExternalInput")
    wvT_d = nc.dram_tensor("wvT", [128, 2, DH], BF16, kind="ExternalInput")
    wpT_d = nc.dram_tensor("wpT", [128, 2, 128], F32R, kind="ExternalInput")
    tqk_d = nc.dram_tensor("tqk", [128, 1], F32, kind="ExternalInput")
    tv_d = nc.dram_tensor("tv", [1, DH], F32, kind="ExternalInput")
    out_d = nc.dram_tensor("out", [2, 128, NSH], F32, kind="ExternalOutput")
    r_d = nc.dram_tensor("r", [128, MT, NSBLOCKS], F32, kind="ExternalOutput")
    dram = (xf_d, xb_d, wqkT_d, wvT_d, wpT_d, tqk_d, tv_d, out_d, r_d)

    with tile.TileContext(nc) as tc:
        with (
            tc.tile_pool(name="const", bufs=1) as const,
            tc.tile_pool(name="work", bufs=3) as work,
            tc.tile_pool(name="outp", bufs=6) as outp,
            tc.tile_pool(name="pgrp", bufs=6) as pgrp,
            tc.tile_pool(name="ps_s", bufs=2, space="PSUM") as ps_s,
            tc.tile_pool(name="ps_xx", bufs=2, space="PSUM") as ps_xx,
            tc.tile_pool(name="ps_pj", bufs=2, space="PSUM") as ps_pj,
        ):
            pools = (const, work, outp, pgrp, ps_s, ps_xx, ps_pj)
            make_tail = None
            for rep in range(reps):
                if probe == "inline":
                    deferred = None
                else:
                    deferred = make_tail(False) if make_tail is not None else None
                make_tail = _emit(nc, pools, dram, pack_s=pack_s, probe=probe,
                                  rep=rep, deferred=deferred)
                if probe == "inline":
                    for piece in make_tail(True):
                        piece()
                    make_tail = None
            if make_tail is not None:
                for piece in make_tail(True):
                    piece()

    nc.compile()
    _CACHE[key] = nc
    return nc


def fold_bn(w, g, b, m, v):
    s = (g / np.sqrt(v + EPS)).astype(np.float32)
    return (w * s[:, None]).astype(np.float32), (b - m * s).astype(np.float32)


def make_in_maps(x, w_qk, g_qk, b_qk, m_qk, v_qk,
                 w_v, g_v, b_v, m_v, v_v, w_p, g_p, b_p, m_p, v_p):
    wqk_f, tqk_f = fold_bn(w_qk, g_qk, b_qk, m_qk, v_qk)   # [16,256], [16]
    wv_f, tv_f = fold_bn(w_v, g_v, b_v, m_v, v_v)          # [128,256], [128]
    wp_f, tp_f = fold_bn(w_p, g_p, b_p, m_p, v_p)          # [256,128], [256]

    # [128, 2, *]: partition dim first, C-half (or out-half) second.
    # wqkT replicated into 4 column groups of 32 (16 used + 16 zero) so the
    # S stage can row-pack 4 concurrent matmuls.
    wqkT_h = wqk_f.T.reshape(2, 128, KD).transpose(1, 0, 2)  # [128, 2, 16]
    wqkT = np.zeros((128, 2, 128), np.float32)
    for g in range(4):
        wqkT[:, :, 32 * g:32 * g + KD] = wqkT_h
    wqkT = np.ascontiguousarray(wqkT)
    wvT = np.ascontiguousarray(
        wv_f.T.reshape(2, 128, DH).transpose(1, 0, 2)).astype(ml_dtypes.bfloat16)
    wpT = np.ascontiguousarray(
        wp_f.T.reshape(128, 2, 128)).astype(np.float32)
    tqk = np.zeros((128, 1), np.float32)
    for g in range(4):
        tqk[32 * g:32 * g + KD, 0] = tqk_f
    tqk = np.ascontiguousarray(tqk)
    tv = tv_f.reshape(1, DH).astype(np.float32)

    xr = x.reshape(B, C, N).astype(np.float32)
    in_maps = []
    for c in range(N_CORES):
        b_, h_ = c // 2, c % 2
        # permute n so this core's half comes first
        if h_ == 0:
            xp = xr[b_]
        else:
            xp = np.concatenate([xr[b_][:, NSH:], xr[b_][:, :NSH]], axis=1)
        xp = np.ascontiguousarray(xp.reshape(2, 128, N).transpose(1, 0, 2))
        in_maps.append({
            "xf": xp.astype(np.float32),
            "xb": xp.astype(ml_dtypes.bfloat16),
            "wqkT": wqkT, "wvT": wvT, "wpT": wpT,
            "tqk": tqk, "tv": tv,
        })
    return in_maps, tp_f


def assemble(results, tp_f):
    """Per-core 'out' [2,128,NSH] + 'r' [128,MT,NSBLOCKS] -> full [B,C,H,W].

    r[n] = R_half0[n] + R_half1[n] by symmetry of P; final epilogue
    out = O_u / r + t_p (relu commutes with the positive 1/r scale, so the
    device applied w_p' @ relu() unnormalized).
    """
    out = np.empty((B, C, N), np.float32)
    for b_ in range(B):
        rv = []
        for h_ in range(2):
            rr = results[2 * b_ + h_]["r"].astype(np.float32)  # [128,MT,2]
            rl = rr.sum(axis=2).T.reshape(-1)  # R[m], local (permuted) order
            if h_ == 1:
                rl = np.concatenate([rl[NSH:], rl[:NSH]])      # unpermute
            rv.append(rl)
        r = rv[0] + rv[1]                                      # [N]
        for h_ in range(2):
            o = results[2 * b_ + h_]["out"].reshape(C, NSH)
            sl = slice(h_ * NSH, (h_ + 1) * NSH)
            out[b_][:, sl] = o / r[sl][None, :] + tp_f[:, None]
    return out.reshape(B, C, H, W)


def kernel(**inputs):
    from concourse.bass_utils import run_bass_kernel_spmd
    from concourse.bass_interp import get_hw_module

    inputs = {k: np.asarray(v) for k, v in inputs.items()}
    inputs.pop("key_v_input_reduction", None)  # unused by the reference
    nc = build_nc()
    in_maps, tp_f = make_in_maps(**inputs)
    old_m = nc.m
    nc.m = get_hw_module(nc.m)
    try:
        res = run_bass_kernel_spmd(nc, in_maps, core_ids=list(range(N_CORES)))
    finally:
        nc.m = old_m
    return assemble(res.results, tp_f)
